# revision 1
# baseline (speedup 1.0000x reference)
"""MultiHeadEMABlock Trainium2 kernel, v3: n-major I/O, PE-side head scaling.

Math (reference):
  h = LayerNorm_c(x[b,c,n] over c) * gamma + beta
  xe[b,n,h,d] = h[b,n,d] * expansion[h,d]
  y = causal damped EMA along n; out[b,d,n] = sum_h y*reduction + x
  => out = x + sum_h rho_h[d] * EMA_{q_h}(z)[n,d],  rho_h = a_h*e_h*r_h*gamma
  beta term added on host (exact, data-independent).

v3 = lessons from v1 (PE-heavy, 140us) and v2 (DVE-bound, 150us):
  - x arrives n-major bf16 (host transpose): LN stats via DVE bn_stats
    (one op), z via one ACT Identity (per-partition scale/bias) — no PE
    stats matmuls, no DVE normalize ops.
  - per-head scaled copies X_h on PE (elementwise engines measured 624+
    ns/tile): transpose z_n back to c-major (4 N=128 matmuls), then v1's
    fused diag(rho) scale+transpose matmuls (4x N=1024 into 2-bank psums).
  - EMA: 8 T_h matmuls per chunk, chunk-PAIR interleaved (2 psum banks,
    avoids the single-bank accumulate serialization that left v2's PE cold).
  - carry: ek matmul on n-major z directly (no xu transpose), bf16 carry
    updates on DVE; rank-8 pmat correction matmul.
  - residual: DVE psum+x add (doubles as psum evacuation), n-major DMA out,
    host transposes back.
"""
import contextlib
import ctypes
import os
import sys
import types

import numpy as np

for _p in ("/root/.axon_site/_ro/trn_rl_repo", "/opt/trn_rl_repo"):
    if _p not in sys.path:
        sys.path.append(_p)

B, C, N, H = 4, 512, 4096, 8
EPS = 1e-5
N_CORES = 8
NHALF = N // 2
CT = C // 128
L = 128  # EMA chunk length


def _install_ntff_shim():
    if "antenv.axon_hooks" in sys.modules:
        return
    holder = {"hook": None}

    def _make(so_path):
        try:
            lib = ctypes.CDLL(so_path)
        except OSError:
            return None
        if not hasattr(lib, "axon_start_nrt_profile"):
            return None
        lib.axon_start_nrt_profile.argtypes = [
            ctypes.POINTER(ctypes.c_int64),
            ctypes.c_size_t,
        ]
        lib.axon_start_nrt_profile.restype = ctypes.c_int64
        lib.axon_stop_nrt_profile.argtypes = [ctypes.c_char_p]
        lib.axon_stop_nrt_profile.restype = ctypes.c_int64

        @contextlib.contextmanager
        def _hook(output_dir, device_ids):
            import jax

            jax.devices()
            if device_ids:
                ids = (ctypes.c_int64 * len(device_ids))(*device_ids)
                rc = lib.axon_start_nrt_profile(ids, len(device_ids))
            else:
                rc = lib.axon_start_nrt_profile(None, 0)
            if rc != 0:
                raise RuntimeError(f"axon_start_nrt_profile rc={rc}")
            try:
                yield
            finally:
                n = lib.axon_stop_nrt_profile(str(output_dir).encode())
                print(f"ntff profile: {n} file(s) -> {output_dir}", file=sys.stderr)

        return _hook

    mod = types.ModuleType("antenv.axon_hooks")
    mod.set_axon_ntff_profile_hook = lambda h: holder.__setitem__("hook", h)
    mod.get_axon_ntff_profile_hook = lambda: holder["hook"]
    sys.modules["antenv.axon_hooks"] = mod
    try:
        import antenv

        antenv.axon_hooks = mod
    except ImportError:
        pass
    holder["hook"] = _make("/opt/axon/libaxon_pjrt.so")


def _patch_ldw_opt():
    """Optionally flip walrus --enable-ldw-opt (A/B via BASS_LDW_OPT=1)."""
    if os.environ.get("BASS_LDW_OPT") != "1":
        return
    import concourse.bass_utils as bu

    if getattr(bu, "_ldw_patched", False):
        return
    orig = bu.run_command

    def patched(cmd, **kw):
        if isinstance(cmd, list):
            cmd = [
                "--enable-ldw-opt=true" if c == "--enable-ldw-opt=false" else c
                for c in cmd
            ]
        return orig(cmd, **kw)

    bu.run_command = patched
    bu._ldw_patched = True


def _split_multiwait(nc, max_waits=1):
    from concourse import mybir

    k = [0]
    for fn in nc.m.functions:
        for blk in fn.blocks:
            out = []
            for inst in blk.instructions:
                si = getattr(inst, "sync_info", None)
                if si is not None and len(si.on_wait) > max_waits:
                    waits = list(si.on_wait)
                    for w in waits[max_waits:]:
                        k[0] += 1
                        out.append(
                            mybir.InstNoOp(
                                name=f"{inst.name}-mw{k[0]}",
                                sync_info=mybir.SyncInfo(on_wait=[w], on_update=[]),
                                bass_nofuse=True,
                                engine=inst.engine,
                            )
                        )
                    inst.sync_info = mybir.SyncInfo(
                        on_wait=waits[:max_waits], on_update=list(si.on_update)
                    )
                out.append(inst)
            blk.instructions[:] = out


# ---------------------------------------------------------------------------
# program builder
# ---------------------------------------------------------------------------
def build_program(W):
    import concourse.bass as bass
    import concourse.tile as tile
    from concourse import mybir

    NW = NHALF + W
    K0 = W // L
    NCH = NW // L
    f32 = mybir.dt.float32
    bf16 = mybir.dt.bfloat16
    Op = mybir.AluOpType
    Act = mybir.ActivationFunctionType

    nc = bass.Bass(
        "TRN2",
        target_bir_lowering=False,
        debug=False,
        enable_asserts=False,
        num_devices=N_CORES,
    )
    # packed layouts: partition-major with wide rows for efficient DMA descriptors
    # misc pack: ident | ek | pmat(rows 0-7) | rho8(rows 0-7) | rhoR for the
    # gpsimd-produced X block (heads 4-7, dtiles 2-3, replicated to 128 rows)
    MW = 128 + H + 128 + C + 4 * 384
    xs_d = nc.dram_tensor("xs", [128, NCH * C], bf16, kind="ExternalInput").ap()
    tm_d = nc.dram_tensor("tmats", [128, H * 128], bf16, kind="ExternalInput").ap()
    w4_d = nc.dram_tensor("w4", [128, H * 512], bf16, kind="ExternalInput").ap()
    mc_d = nc.dram_tensor("miscc", [128, MW], bf16, kind="ExternalInput").ap()
    ql_d = nc.dram_tensor("qlcol", [H, 1], f32, kind="ExternalInput").ap()
    out_d = nc.dram_tensor("out_t", [NHALF, C], f32, kind="ExternalOutput").ap()

    with tile.TileContext(nc) as tc:
        with contextlib.ExitStack() as ctx:
            pers = ctx.enter_context(tc.tile_pool(name="pers", bufs=1))
            x_pool = ctx.enter_context(tc.tile_pool(name="xp", bufs=4))
            z_pool = ctx.enter_context(tc.tile_pool(name="zp", bufs=4))
            zc_pool = ctx.enter_context(tc.tile_pool(name="zcp", bufs=3))
            xh_pool = ctx.enter_context(tc.tile_pool(name="xhp", bufs=4))
            st_pool = ctx.enter_context(tc.tile_pool(name="stp", bufs=6))
            cr_pool = ctx.enter_context(tc.tile_pool(name="crp", bufs=3))
            o_pool = ctx.enter_context(tc.tile_pool(name="op", bufs=3))
            ps_pool = ctx.enter_context(tc.tile_pool(name="ps", bufs=1, space="PSUM"))

            # ---- constants: halo x first (unblocks the LN front), then one
            # packed DMA for all small consts, then the big T8/W4 loads ----
            x_halo = x_pool.tile([128, K0 * C], bf16, tag="x", name="xhalo")
            nc.sync.dma_start(out=x_halo[:], in_=xs_d[:, 0 : K0 * C])
            mbig = pers.tile([128, MW], bf16, tag="mbig")
            nc.sync.dma_start(out=mbig[:], in_=mc_d)
            ident = mbig[:, 0:128]
            ek = mbig[:, 128 : 128 + H]
            pmat = mbig[0:H, 128 + H : 256 + H]
            rho8 = mbig[0:H, 256 + H : 256 + H + C]
            rrg = [mbig[:, 776 + hp * 384 : 776 + (hp + 1) * 384] for hp in range(4)]
            T8big = pers.tile([128, H * 128], bf16, tag="T8big")
            nc.scalar.dma_start(out=T8big[:], in_=tm_d)
            T8 = [T8big[:, h * 128 : (h + 1) * 128] for h in range(H)]
            W4big = pers.tile([128, H * 512], bf16, tag="W4big")
            nc.sync.dma_start(out=W4big[0:64, :], in_=w4_d[0:64, :])
            nc.scalar.dma_start(out=W4big[64:128, :], in_=w4_d[64:128, :])
            W4 = [W4big[:, i * 512 : (i + 1) * 512] for i in range(H)]
            qlc = pers.tile([H, 1], f32, tag="qlc")
            nc.sync.dma_start(out=qlc[:], in_=ql_d)
            epsb = pers.tile([128, 1], f32, tag="eps")
            nc.gpsimd.memset(epsb[:], EPS)

            c_cur = cr_pool.tile([H, C], bf16, tag="carry")
            nc.gpsimd.memset(c_cur[:], 0.0)

            def dma_x2(kk):
                """One DMA covering chunks kk, kk+1 (2 KB/partition lines)."""
                x2 = x_pool.tile([128, 2 * C], bf16, tag="x", name=f"x{kk}")
                nc.sync.dma_start(out=x2[:], in_=xs_d[:, kk * C : (kk + 2) * C])
                return x2

            def ln_front(k, x_k, st6=None):
                """LN stats + z_n from an x view; returns z_k.  If st6 is given
                (precomputed pair bn_stats slice), skips the bn_stats op."""
                if st6 is None:
                    st6 = st_pool.tile([128, 6], f32, tag="st6")
                    nc.vector.bn_stats(out=st6[:], in_=x_k)
                    st6 = st6[:]
                mv = st_pool.tile([128, 2], f32, tag="mv")
                nc.vector.bn_aggr(out=mv[:], in_=st6)
                lnv = st_pool.tile([128, 1], f32, tag="lnv")
                nc.scalar.activation(out=lnv[:], in_=mv[:, 1:2], func=Act.Ln,
                                     bias=epsb[:])
                rstd = st_pool.tile([128, 1], f32, tag="rstd")
                nc.scalar.activation(out=rstd[:], in_=lnv[:], func=Act.Exp, scale=-0.5)
                z_k = z_pool.tile([128, C], bf16, tag="z", name=f"z{k}")
                if k % 2 == 0:  # even chunks: ACT Identity (needs -mean*rstd bias)
                    negms = st_pool.tile([128, 1], f32, tag="negms")
                    nc.vector.tensor_scalar(
                        out=negms[:], in0=mv[:, 0:1], scalar1=rstd[:, 0:1],
                        scalar2=-1.0, op0=Op.mult, op1=Op.mult,
                    )
                    nc.scalar.activation(
                        out=z_k[:], in_=x_k, func=Act.Identity,
                        bias=negms[:, 0:1], scale=rstd[:, 0:1],
                    )
                else:  # odd chunks: DVE (x - mean) * rstd, balances ACT load
                    nc.vector.tensor_scalar(
                        out=z_k[:], in0=x_k, scalar1=mv[:, 0:1],
                        scalar2=rstd[:, 0:1], op0=Op.subtract, op1=Op.mult,
                    )
                return z_k

            def e_mm(z_k):
                e_ps = ps_pool.tile([H, C], f32, tag="misc", bufs=2)
                nc.tensor.matmul(out=e_ps[:], lhsT=ek, rhs=z_k[:], start=True,
                                 stop=True)
                return e_ps

            def carry_update(c_prev, e_ps):
                erho = cr_pool.tile([H, C], bf16, tag="erho")
                nc.vector.tensor_tensor(out=erho[:], in0=e_ps[:], in1=rho8,
                                        op=Op.mult)
                c_nxt = cr_pool.tile([H, C], bf16, tag="carry")
                nc.vector.scalar_tensor_tensor(
                    out=c_nxt[:], in0=c_prev[:], scalar=qlc[:, 0:1], in1=erho[:],
                    op0=Op.mult, op1=Op.add,
                )
                return c_nxt

            def transp(z_k):
                """z_n -> (PE transpose) -> zc (c-major bf16 SBUF)."""
                zt_ps = ps_pool.tile([128, C], f32, tag="misc", bufs=2)
                for dt in range(CT):
                    nc.tensor.matmul(
                        out=zt_ps[:, dt * 128 : (dt + 1) * 128],
                        lhsT=z_k[:, dt * 128 : (dt + 1) * 128], rhs=ident,
                        start=True, stop=True,
                    )
                zc = zc_pool.tile([128, C], bf16, tag="zc")
                nc.scalar.activation(out=zc[:], in_=zt_ps[:], func=Act.Copy)
                return zc

            def make_xh_gen(z_k, zc, k, out):
                """Generator: emits the diag matmuls + evacs for one chunk,
                yielding after each matmul so the caller can interleave them
                with T-block matmuls.  The (g=1,dp=1) quarter is produced on
                the otherwise-idle GpSimd (SBUF-only z*rho, a full round of
                slack), saving 2 PE matmuls and one ACT evacuation."""
                xh = xh_pool.tile([128, H * 512], bf16, tag="xh", name=f"xh{k}")
                out.append(xh[:].rearrange("p (g dt hp jj) -> p g dt hp jj",
                                           g=2, dt=CT, hp=4))
                zg = z_k[:, 128:512].rearrange("p (dt jj) -> p dt jj", dt=3)
                for hp in range(4):
                    nc.gpsimd.tensor_tensor(
                        out=xh[:, 2560:4096].rearrange(
                            "p (dt hp jj) -> p dt hp jj", dt=3, hp=4)[:, :, hp, :],
                        in0=zg,
                        in1=rrg[hp].rearrange("p (dt jj) -> p dt jj", dt=3),
                        op=Op.mult,
                    )
                for g in range(2):
                    for dp in range(2):
                        if g == 1 and dp == 1:
                            continue
                        gp_half = g == 1 and dp == 0  # dt=1 produced on gpsimd
                        sp = ps_pool.tile([128, 1024], f32, tag="xps", bufs=2)
                        for dd in range(1 if gp_half else 2):
                            dt = dp * 2 + dd
                            nc.tensor.matmul(
                                out=sp[:, dd * 512 : (dd + 1) * 512],
                                lhsT=zc[:, dt * 128 : (dt + 1) * 128],
                                rhs=W4[g * CT + dt],
                                start=True, stop=True,
                            )
                            yield
                        w = 512 if gp_half else 1024
                        dst = xh[:, g * 2048 + dp * 1024 : g * 2048 + dp * 1024 + w]
                        # 3/1 ACT/DVE evac split (DVE holds the carry/resid chains)
                        on_act = (g + dp) % 2 == 0 or (k + g) % 2 == 0
                        srcv = sp[:, 0:w]
                        if on_act:
                            nc.scalar.activation(out=dst, in_=srcv, func=Act.Copy)
                        else:
                            nc.vector.tensor_scalar(
                                out=dst, in0=srcv, scalar1=1.0, scalar2=None,
                                op0=Op.mult,
                            )

            # ---- halo chunks: carries only (x_halo preloaded above) ----
            for j in range(K0):
                z_k = ln_front(j, x_halo[:, j * C : (j + 1) * C])
                e_ps = e_mm(z_k)
                c_cur = carry_update(c_cur, e_ps)

            def interleave(gens):
                gens = [iter(g) for g in gens]
                alive = True
                while alive:
                    alive = False
                    for g in gens:
                        try:
                            next(g)
                            alive = True
                            yield
                        except StopIteration:
                            pass

            def stage_c(pair, fronts, xh_iter):
                psums = []
                for h in range(H):  # pair-interleaved: T8[h] stationary reuse
                    g, hp = divmod(h, 4)
                    for i, k in enumerate(pair):
                        if h == 0:
                            psums.append(ps_pool.tile([128, 512], f32, tag="ema",
                                                      bufs=2, name=f"emaps{k}"))
                        nc.tensor.matmul(
                            out=psums[i][:], lhsT=T8[h],
                            rhs=fronts[i][1][:, g, :, hp, :],
                            start=(h == 0), stop=False,
                        )
                    if h >= 2:  # feed next pair's scale-matmuls between T heads
                        next(xh_iter, None)
                        next(xh_iter, None)
                for i, k in enumerate(pair):
                    nc.tensor.matmul(out=psums[i][:], lhsT=pmat,
                                     rhs=fronts[i][2][:], start=False, stop=True)
                for i, k in enumerate(pair):
                    o_sb = o_pool.tile([128, C], f32, tag="osb")
                    nc.vector.tensor_tensor(out=o_sb[:], in0=psums[i][:],
                                            in1=fronts[i][0], op=Op.add)
                    ko = k - K0
                    nc.sync.dma_start(out=out_d[ko * L : (ko + 1) * L, :], in_=o_sb[:])

            # ---- main chunks: software-pipelined pairs.  Per round: LN+transpose
            # fronts of pair P, then the T-block of pair P-1 with pair P's
            # scale-matmuls interleaved into it (PE stays fed while ACT/DVE
            # evacuate), then E/carries of pair P. ----
            cr_holder = [c_cur]
            ks = list(range(K0, NCH))
            pairs = [ks[i : i + 2] for i in range(0, len(ks), 2)]
            pending = None
            for pi, pair in enumerate(pairs):
                mids = []
                xh_views = []
                x2 = dma_x2(pair[0])
                for j, k in enumerate(pair):
                    x_k = x2[:, j * C : (j + 1) * C]
                    z_k = ln_front(k, x_k)
                    mids.append((k, x_k, z_k, transp(z_k)))
                xh_iter = interleave(
                    [make_xh_gen(z_k, zc, k, xh_views) for k, _, z_k, zc in mids]
                )
                if pending is not None:
                    stage_c(*pending, xh_iter)
                for _ in xh_iter:  # emit any remaining scale-matmuls/evacs
                    pass
                fronts = []
                for i, (k, x_k, z_k, zc) in enumerate(mids):
                    cr_in = cr_holder[0]
                    if k < NCH - 1:
                        e_ps = e_mm(z_k)
                        cr_holder[0] = carry_update(cr_holder[0], e_ps)
                    fronts.append((x_k, xh_views[i], cr_in))
                if pi == 0:
                    # first pair: run its T-block immediately (lag 0) so PE
                    # starts a full round earlier during pipeline fill
                    stage_c(pair, fronts, iter(()))
                else:
                    pending = (pair, fronts)
            stage_c(*pending, iter(()))
    return nc


def _host_params(ln_gamma, ln_beta, expansion, reduction, alphas, dampen_factors):
    import ml_dtypes

    a = 1.0 / (1.0 + np.exp(-alphas.astype(np.float64)))
    q = (1.0 - a) / (1.0 + np.exp(-dampen_factors.astype(np.float64)))
    qmax = float(q.max())
    W = L
    while qmax**W > 1e-12 and W < NHALF:
        W += L
    rho = (
        a[:, None]
        * expansion.astype(np.float64)
        * reduction.astype(np.float64)
        * ln_gamma.astype(np.float64)[None, :]
    )  # [H, C]
    bf = ml_dtypes.bfloat16
    ii, jj = np.meshgrid(np.arange(L), np.arange(L), indexing="ij")
    tmats = np.zeros((H * 128, 128), bf)
    for h in range(H):
        M = np.where(ii >= jj, q[h] ** np.maximum(ii - jj, 0), 0.0)  # T_h[i,j]
        tmats[h * 128 : (h + 1) * 128, :] = M.T.astype(bf)  # lhsT[j,i]
    w4 = np.zeros((H * 128, 512), bf)
    for g in range(2):
        for dt in range(CT):
            blk = np.zeros((128, 512))
            for hp in range(4):
                h = g * 4 + hp
                blk[:, hp * 128 : (hp + 1) * 128] = np.diag(rho[h, dt * 128 : (dt + 1) * 128])
            w4[(g * CT + dt) * 128 : (g * CT + dt + 1) * 128, :] = blk.astype(bf)
    ek = np.zeros((128, H), bf)
    for h in range(H):
        ek[:, h] = (q[h] ** (L - 1 - np.arange(L))).astype(bf)
    pmat = np.zeros((H, 128), bf)
    for h in range(H):
        pmat[h, :] = (q[h] ** (np.arange(L) + 1.0)).astype(bf)
    ident = np.eye(128, dtype=bf)
    rho8 = rho.astype(bf)
    qlcol = (q**L).astype(np.float32).reshape(H, 1)
    # pack tile-per-128-rows layouts into partition-major wide-row layouts
    tmats = np.ascontiguousarray(
        tmats.reshape(H, 128, 128).transpose(1, 0, 2).reshape(128, H * 128)
    )
    w4 = np.ascontiguousarray(
        w4.reshape(H, 128, 512).transpose(1, 0, 2).reshape(128, H * 512)
    )
    # one packed tile for the small consts: ident | ek | pmat | rho8 | rhoR(g1dp1)
    miscc = np.zeros((128, 128 + H + 128 + C + 4 * 384), bf)
    miscc[:, 0:128] = ident
    miscc[:, 128 : 128 + H] = ek
    miscc[0:H, 128 + H : 256 + H] = pmat
    miscc[0:H, 256 + H : 256 + H + C] = rho8
    for hp in range(4):
        miscc[:, 776 + hp * 384 : 776 + (hp + 1) * 384] = np.broadcast_to(
            rho8[4 + hp, 128:512], (128, 384)
        )
    consts = dict(tmats=tmats, w4=w4, miscc=miscc, qlcol=qlcol)
    return a, q, W, consts


def _beta_term(ln_beta, expansion, reduction, a, q):
    if not np.any(ln_beta):
        return None
    n_idx = np.arange(N, dtype=np.float64)
    Cn = a[:, None] * (1.0 - q[:, None] ** (n_idx[None, :] + 1.0)) / (1.0 - q[:, None])
    w = (
        expansion.astype(np.float64)
        * reduction.astype(np.float64)
        * ln_beta.astype(np.float64)[None, :]
    )
    return np.einsum("hc,hn->cn", w, Cn).astype(np.float32)


def _make_in_maps(x, W, consts):
    import ml_dtypes

    bf = ml_dtypes.bfloat16
    NW = NHALF + W
    NCH = NW // L
    in_maps = []
    for core in range(N_CORES):
        b, half = divmod(core, 2)
        xs = np.zeros((NW, C), bf)
        s = half * NHALF - W
        if s < 0:
            xs[W:, :] = x[b, :, :NHALF].T.astype(bf)
        else:
            xs[:, :] = x[b, :, s : s + NW].T.astype(bf)
        # pack chunk-major: xs2[p, k*C + c] = xs[k*128 + p, c]
        xs2 = np.ascontiguousarray(
            xs.reshape(NCH, 128, C).transpose(1, 0, 2).reshape(128, NCH * C)
        )
        in_maps.append(dict(consts, xs=xs2))
    return in_maps


def kernel(x, ln_gamma, ln_beta, expansion, reduction, alphas, dampen_factors,
           trace=False):
    _install_ntff_shim()
    _patch_ldw_opt()
    from concourse.bass_utils import run_bass_kernel_spmd
    from concourse.bass_interp import get_hw_module

    x = np.asarray(x, np.float32)
    a, q, W, consts = _host_params(
        np.asarray(ln_gamma), np.asarray(ln_beta), np.asarray(expansion),
        np.asarray(reduction), np.asarray(alphas), np.asarray(dampen_factors),
    )
    nc = build_program(W)
    _split_multiwait(nc)
    nc.m = get_hw_module(nc.m)

    in_maps = _make_in_maps(x, W, consts)
    res = run_bass_kernel_spmd(
        nc, in_maps, core_ids=list(range(N_CORES)), trace=trace
    )

    out = np.empty((B, C, N), np.float32)
    for core in range(N_CORES):
        b, half = divmod(core, 2)
        out[b, :, half * NHALF : (half + 1) * NHALF] = res.results[core]["out_t"].T
    bt = _beta_term(
        np.asarray(ln_beta), np.asarray(expansion), np.asarray(reduction), a, q
    )
    if bt is not None:
        out += bt[None]
    if trace:
        kernel.last_results = res
    return out

